# revision 1
# baseline (speedup 1.0000x reference)
"""GQA attention block (B=1, S=2048, D=2048, H=32, G=8, HD=64) on 8 trn2 cores.

Sharding: tensor-parallel over heads/KV-groups. Core c owns q-heads
4c..4c+3 and KV group c. Wq/Wk/Wv column-parallel, Wo row-parallel;
each core computes a partial [S, D] output, host sums the 8 partials.

Per-core dataflow (all matmuls bf16, stats f32):
  xT [d,s] resident in SBUF.
  qkv proj -> psum [s,384] (q 4x64 | k 64 | v 64), RMSNorm+RoPE on DVE
  (rsqrt via Newton, rope via host-premodified cos/sin tables),
  PE-transpose q/k to [hd, s]; v stays [s, hd] (+ ones column for the
  softmax denominator).
  gate proj directly in [e, s] layout; sigmoid via tanh (single ACT
  table set with exp: "exp_and_others").
  scoresT[sk,sq] = kT.T @ qT per head; probs = exp(scale*scores) with
  causal masking (skip blocks above diagonal, triangular mask multiply
  on diagonal blocks); ctxT[hd,sq] (+den row) = [v|1].T @ probsT.
  ctxg = ctx * (1+tanh(g/2)) * (0.5/den)  (den recip broadcast via
  ones-matmul on PE).
  out[s,dout] = ctxg.T @ woT, partials written bf16.
"""

import numpy as np
import ml_dtypes

import concourse.bass as bass
import concourse.tile as tile
from concourse import bacc, mybir
from concourse.bass_utils import run_bass_kernel_spmd
from concourse.masks import make_identity

BF16 = mybir.dt.bfloat16
F32 = mybir.dt.float32
NBF = ml_dtypes.bfloat16

S = 2048
D = 2048
H = 32
G = 8
HD = 64
NCORE = 8
NHL = H // NCORE          # 4 q heads per core
EL = NHL * HD             # 256 local q (and gate, and ctx) features
QK = EL + HD              # 320: q + k features
QKV = QK + HD             # 384: q + k + v
P = 128
NS = S // P               # 16 s-tiles
ND = D // P               # 16 d-tiles
SQ = 512
NSQ = S // SQ             # 4 sq slices
NB = QK // HD             # 5 (hd,) blocks in the q|k strip
SCALE = HD ** -0.5
EPS = 1e-6


def _v(ap, dims, extra_offset=0):
    """Reshape the free dims of a 2D AP into `dims` ([step, count] pairs),
    keeping the partition dim."""
    return bass.AP(
        tensor=ap.tensor,
        offset=ap.offset + extra_offset,
        ap=[list(ap.ap[0])] + [list(d) for d in dims],
    )


def _mk(pool, shape, dtype, tag):
    return pool.tile(shape, dtype, tag=tag, name=tag)


def build_nc():
    nc = bacc.Bacc("TRN2", target_bir_lowering=False, debug=False,
                   num_devices=NCORE)

    xt = nc.dram_tensor("xt", [D, S], BF16, kind="ExternalInput").ap()
    wqkv = nc.dram_tensor("wqkv", [D, QKV], BF16, kind="ExternalInput").ap()
    wg = nc.dram_tensor("wg", [D, EL], BF16, kind="ExternalInput").ap()
    wo = nc.dram_tensor("wo", [EL, D], BF16, kind="ExternalInput").ap()
    cos5 = nc.dram_tensor("cos5", [S, QK], BF16, kind="ExternalInput").ap()
    sin5 = nc.dram_tensor("sin5", [S, QK], BF16, kind="ExternalInput").ap()
    tri = nc.dram_tensor("tri", [P, P], BF16, kind="ExternalInput").ap()
    out = nc.dram_tensor("out", [S, D], BF16, kind="ExternalOutput").ap()

    with tile.TileContext(nc) as tc:
        with (
            tc.tile_pool(name="persist", bufs=1) as pp,
            tc.tile_pool(name="work", bufs=2) as wp,
            tc.tile_pool(name="stats", bufs=3) as sp,
            tc.tile_pool(name="probs", bufs=4) as prp,
            tc.tile_pool(name="outc", bufs=4) as ocp,
            tc.tile_pool(name="psum", bufs=8, space="PSUM") as psp,
        ):
            # ---- persistent loads ----
            xts = []
            for i in range(ND):
                t = _mk(pp, [P, S], BF16, f"xt{i}")
                nc.sync.dma_start(out=t, in_=xt[i * P:(i + 1) * P, :])
                xts.append(t)
            wqkvs = []
            for i in range(ND):
                t = _mk(pp, [P, QKV], BF16, f"wqkv{i}")
                nc.sync.dma_start(out=t, in_=wqkv[i * P:(i + 1) * P, :])
                wqkvs.append(t)
            wgs = []
            for i in range(ND):
                t = _mk(pp, [P, EL], BF16, f"wg{i}")
                nc.sync.dma_start(out=t, in_=wg[i * P:(i + 1) * P, :])
                wgs.append(t)
            wos = []
            for e in range(2):
                t = _mk(pp, [P, D], BF16, f"wo{e}")
                nc.sync.dma_start(out=t, in_=wo[e * P:(e + 1) * P, :])
                wos.append(t)
            coss, sins = [], []
            for j in range(NS):
                tc_ = _mk(pp, [P, QK], BF16, f"cos{j}")
                nc.sync.dma_start(out=tc_, in_=cos5[j * P:(j + 1) * P, :])
                coss.append(tc_)
                ts_ = _mk(pp, [P, QK], BF16, f"sin{j}")
                nc.sync.dma_start(out=ts_, in_=sin5[j * P:(j + 1) * P, :])
                sins.append(ts_)
            tri_sb = _mk(pp, [P, P], BF16, "tri")
            nc.sync.dma_start(out=tri_sb, in_=tri)
            ident = _mk(pp, [P, P], BF16, "ident")
            make_identity(nc, ident)
            halfones = _mk(pp, [1, P], F32, "halfones")
            nc.vector.memset(halfones, 0.5)

            # persistent intermediate tensors
            qth = [[_mk(pp, [HD, SQ], BF16, f"qt{h}_{q}") for q in range(NSQ)]
                   for h in range(NHL)]
            kts = [_mk(pp, [HD, SQ], BF16, f"kt{q}") for q in range(NSQ)]
            vs = [_mk(pp, [P, HD + 1], BF16, f"v{j}") for j in range(NS)]
            gus = [[_mk(pp, [P, SQ], BF16, f"gu{p}_{q}") for q in range(NSQ)]
                   for p in range(2)]
            ctxgs = [[_mk(pp, [P, SQ], BF16, f"cg{p}_{q}") for q in range(NSQ)]
                     for p in range(2)]

            # ---- phase 1: qkv projection + norm + rope + transpose ----
            for j in range(NS):
                ps_qkv = _mk(psp, [P, QKV], F32, "ps")
                for i in range(ND):
                    nc.tensor.matmul(
                        ps_qkv, xts[i][:, j * P:(j + 1) * P], wqkvs[i],
                        start=(i == 0), stop=(i == ND - 1))
                # v (+ ones column) straight to SBUF
                nc.vector.tensor_copy(out=vs[j][:, :HD], in_=ps_qkv[:, QK:QKV])
                nc.vector.memset(vs[j][:, HD:HD + 1], 1.0)
                # RMS stats over each of the 5 (hd,) blocks of q|k
                qk = ps_qkv[:, :QK]
                qk5 = _v(qk, [[HD, NB], [1, HD]])
                sqr = _mk(wp, [P, QK], F32, "sqr")
                nc.scalar.activation(sqr, qk,
                                     mybir.ActivationFunctionType.Square)
                ss = _mk(sp, [P, NB], F32, "ss")
                nc.vector.tensor_reduce(
                    ss, _v(sqr, [[HD, NB], [1, HD]]),
                    axis=mybir.AxisListType.X, op=mybir.AluOpType.add)
                # m = mean + eps;  r = rsqrt(m) via poly seed + 3 Newton steps
                m = _mk(sp, [P, NB], F32, "m")
                nc.vector.tensor_scalar(m, ss, 1.0 / HD, EPS,
                                        mybir.AluOpType.mult,
                                        mybir.AluOpType.add)
                # quadratic minimax seed on clamped m, then 3 Newton steps
                mc = _mk(sp, [P, NB], F32, "mc")
                nc.vector.tensor_scalar(mc, m, 5.5, 0.45,
                                        mybir.AluOpType.min,
                                        mybir.AluOpType.max)
                m2 = _mk(sp, [P, NB], F32, "m2")
                nc.vector.tensor_mul(m2, mc, mc)
                lin = _mk(sp, [P, NB], F32, "lin")
                nc.vector.tensor_scalar(lin, mc, -0.48330447, 1.51774376,
                                        mybir.AluOpType.mult,
                                        mybir.AluOpType.add)
                y = _mk(sp, [P, NB], F32, "y")
                nc.vector.scalar_tensor_tensor(y, m2, 0.0534932, lin,
                                               mybir.AluOpType.mult,
                                               mybir.AluOpType.add)
                ytmp = _mk(sp, [P, NB], F32, "ytmp")
                for _ in range(3):
                    nc.vector.tensor_mul(ytmp, y, y)              # y^2
                    nc.vector.tensor_mul(ytmp, ytmp, m)           # m y^2
                    nc.vector.tensor_scalar(ytmp, ytmp, -0.5, 1.5,
                                            mybir.AluOpType.mult,
                                            mybir.AluOpType.add)
                    nc.vector.tensor_mul(y, y, ytmp)
                # qkn = qk * r (broadcast r over hd)
                qkn = _mk(wp, [P, QK], F32, "qkn")
                nc.vector.tensor_mul(
                    _v(qkn, [[HD, NB], [1, HD]]), qk5,
                    _v(y[:, :], [[1, NB], [0, HD]]))
                # rope: out = qkn*cos5 + rot(qkn)*sin5  (sin pre-negated on
                # the first half on host; cos/sin already include 1+norm_w)
                t1 = _mk(wp, [P, QK], F32, "t1")
                nc.vector.tensor_mul(t1, qkn, coss[j])
                t2 = _mk(wp, [P, QK], F32, "t2")
                rot = _v(qkn[:, :], [[HD, NB], [-32, 2], [1, 32]],
                         extra_offset=32)
                nc.vector.tensor_mul(
                    _v(t2, [[HD, NB], [32, 2], [1, 32]]), rot,
                    _v(sins[j][:, :], [[HD, NB], [32, 2], [1, 32]]))
                qkr = _mk(wp, [P, QK], BF16, "qkr")
                nc.vector.tensor_add(qkr, t1, t2)
                # transpose q packs and k into [hd, s] layout
                jq, jc = j // 4, (j % 4) * P
                for p in range(2):
                    pt = _mk(psp, [P, P], BF16, "ps")
                    nc.tensor.transpose(pt, qkr[:, p * P:(p + 1) * P], ident)
                    nc.vector.tensor_copy(out=qth[2 * p][jq][:, jc:jc + P],
                                          in_=pt[:HD, :])
                    nc.vector.tensor_copy(out=qth[2 * p + 1][jq][:, jc:jc + P],
                                          in_=pt[HD:, :])
                ptk = _mk(psp, [HD, P], BF16, "ps")
                nc.tensor.transpose(ptk, qkr[:, 2 * P:2 * P + HD], ident)
                nc.vector.tensor_copy(out=kts[jq][:, jc:jc + P], in_=ptk)

            # ---- phase 1b: gate projection + tanh ----
            for p in range(2):
                for q in range(NSQ):
                    ps_g = _mk(psp, [P, SQ], F32, "ps")
                    for i in range(ND):
                        nc.tensor.matmul(
                            ps_g, wgs[i][:, p * P:(p + 1) * P],
                            xts[i][:, q * SQ:(q + 1) * SQ],
                            start=(i == 0), stop=(i == ND - 1))
                    # u = 1 + tanh(g/2) = 2*sigmoid(g)
                    nc.scalar.activation(gus[p][q], ps_g,
                                         mybir.ActivationFunctionType.Tanh,
                                         scale=0.5)
                    nc.vector.tensor_scalar_add(gus[p][q], gus[p][q], 1.0)

            # ---- phase 2: attention per head / sq slice ----
            for h in range(NHL):
                hp, ho = h // 2, (h % 2) * HD
                for q in range(NSQ):
                    ps_ctx = _mk(psp, [HD + 1, SQ], F32, "ps")
                    nks = 4 * q + 4   # sk tiles 0..nks-1 intersect causally
                    for jk in range(nks):
                        dlt = jk - 4 * q
                        c0 = max(dlt, 0) * P  # cols < c0 fully masked
                        ps_s = _mk(psp, [P, SQ], F32, "ps")
                        nc.tensor.matmul(
                            ps_s[:, c0:],
                            kts[jk // 4][:, (jk % 4) * P:(jk % 4 + 1) * P],
                            qth[h][q][:, c0:],
                            start=True, stop=True)
                        pr = _mk(prp, [P, SQ], BF16, "pr")
                        if c0 > 0:
                            nc.vector.memset(pr[:, :c0], 0.0)
                        nc.scalar.activation(
                            pr[:, c0:], ps_s[:, c0:],
                            mybir.ActivationFunctionType.Exp, scale=SCALE)
                        if dlt >= 0:
                            nc.vector.tensor_mul(
                                pr[:, c0:c0 + P], pr[:, c0:c0 + P], tri_sb)
                        nc.tensor.matmul(ps_ctx, vs[jk], pr,
                                         start=(jk == 0), stop=(jk == nks - 1))
                    # denominator: recip + broadcast via PE (x0.5 folded in)
                    denr = _mk(sp, [1, SQ], F32, "denr")
                    nc.vector.reciprocal(denr, ps_ctx[HD:HD + 1, :])
                    ps_db = _mk(psp, [P, SQ], F32, "ps")
                    nc.tensor.matmul(ps_db, halfones, denr,
                                     start=True, stop=True)
                    # ctxg = ctx * u * (0.5/den)
                    tmpu = _mk(wp, [HD, SQ], F32, "tmpu")
                    nc.vector.tensor_mul(tmpu, ps_ctx[:HD, :],
                                         gus[hp][q][ho:ho + HD, :])
                    nc.vector.tensor_mul(
                        ctxgs[hp][q][ho:ho + HD, :], tmpu, ps_db[:HD, :])

            # ---- phase 3: output projection ----
            for j in range(NS):
                jq, jc = j // 4, (j % 4) * P
                for n in range(NSQ):
                    ps_o = _mk(psp, [P, SQ], F32, "ps")
                    for e in range(2):
                        nc.tensor.matmul(
                            ps_o, ctxgs[e][jq][:, jc:jc + P],
                            wos[e][:, n * SQ:(n + 1) * SQ],
                            start=(e == 0), stop=(e == 1))
                    oc = _mk(ocp, [P, SQ], BF16, "oc")
                    nc.scalar.copy(oc, ps_o)
                    nc.sync.dma_start(
                        out=out[j * P:(j + 1) * P, n * SQ:(n + 1) * SQ],
                        in_=oc)

    nc.compile()
    return nc


def prep_inputs(x, cos, sin, Wq, Wk, Wv, Wo, q_norm_w, k_norm_w):
    """Host-side shard + layout prep. Returns per-core input maps."""
    xtn = np.ascontiguousarray(x.reshape(S, D).T).astype(NBF)

    # rope tables with (1 + norm_w) folded in, k-block appended, and the
    # sin first-half pre-negated (so rope is out = q*cos5 + rot(q)*sin5
    # with rot(q) = [q2, q1])
    half = HD // 2
    wq1 = (1.0 + q_norm_w).astype(np.float32)
    wk1 = (1.0 + k_norm_w).astype(np.float32)

    def rotw(w):
        return np.concatenate([w[half:], w[:half]])

    sin_m = sin.copy()
    sin_m[:, :half] = -sin_m[:, :half]
    cos_q = cos * wq1
    cos_k = cos * wk1
    sin_q = sin_m * rotw(wq1)
    sin_k = sin_m * rotw(wk1)
    cos5 = np.concatenate([np.tile(cos_q, (1, NHL)), cos_k], axis=1)
    sin5 = np.concatenate([np.tile(sin_q, (1, NHL)), sin_k], axis=1)
    cos5 = np.ascontiguousarray(cos5).astype(NBF)
    sin5 = np.ascontiguousarray(sin5).astype(NBF)

    tri = np.triu(np.ones((P, P), dtype=np.float32)).astype(NBF)  # p<=f

    Wqh = Wq.reshape(H, 2 * HD, D)
    in_maps = []
    for c in range(NCORE):
        hs = slice(NHL * c, NHL * (c + 1))
        wq_c = Wqh[hs, :HD, :].reshape(EL, D)       # q rows, 4 heads
        wgt_c = Wqh[hs, HD:, :].reshape(EL, D)      # gate rows
        wk_c = Wk[HD * c:HD * (c + 1), :]
        wv_c = Wv[HD * c:HD * (c + 1), :]
        wqkv_c = np.concatenate([wq_c, wk_c, wv_c], axis=0)  # [384, D]
        in_maps.append({
            "xt": xtn,
            "wqkv": np.ascontiguousarray(wqkv_c.T).astype(NBF),
            "wg": np.ascontiguousarray(wgt_c.T).astype(NBF),
            "wo": np.ascontiguousarray(
                Wo[:, EL * c:EL * (c + 1)].T).astype(NBF),
            "cos5": cos5,
            "sin5": sin5,
            "tri": tri,
        })
    return in_maps


_NC_CACHE = {}


def get_nc():
    if "nc" not in _NC_CACHE:
        _NC_CACHE["nc"] = build_nc()
    return _NC_CACHE["nc"]


def run(in_maps, trace=False, **kw):
    nc = get_nc()
    return run_bass_kernel_spmd(nc, in_maps, list(range(NCORE)),
                                trace=trace, **kw)


def kernel(x, mask, cos, sin, Wq, Wk, Wv, Wo, q_norm_w, k_norm_w):
    in_maps = prep_inputs(np.asarray(x, dtype=np.float32), np.asarray(cos),
                          np.asarray(sin), np.asarray(Wq), np.asarray(Wk),
                          np.asarray(Wv), np.asarray(Wo),
                          np.asarray(q_norm_w), np.asarray(k_norm_w))
    res = run(in_maps)
    acc = np.zeros((S, D), dtype=np.float32)
    for r in res.results:
        acc += np.asarray(r["out"], dtype=np.float32)
    return acc.reshape(1, S, D)



# revision 17
# speedup vs baseline: 2.0299x; 2.0299x over previous
"""GQA attention block (B=1, S=2048, D=2048, H=32, G=8, HD=64) on 8 trn2 cores.

Sharding: tensor-parallel over heads/KV-groups. Core c owns q-heads
4c..4c+3 and KV group c. Wq/Wk/Wv column-parallel, Wo row-parallel;
each core computes a partial [S, D] output, host sums the 8 partials.

v2 schedule (vs v1): gate projection runs first, i-outer across all 8
psum banks so PE consumes x tiles as DMA delivers them; qkv projection
in 3 i-outer waves of 6 banks with the RMSNorm Newton chain batched
per wave and PE transposes of wave w interleaved behind wave w+1's
matmuls; attention packs the two heads of a pair into one PE pass via
row tiling (K=64 each at base partitions 0/64), exp over both heads'
scores in one ACT op, causally-trimmed ctx matmuls pipelined one step
behind scores; softmax denominator is PE-broadcast first then
reciprocal'd on all 128 lanes.
"""

import numpy as np
import ml_dtypes

import concourse.bass as bass
import concourse.tile as tile
from concourse import bacc, mybir
from concourse.bass_utils import run_bass_kernel_spmd
from concourse.masks import make_identity

BF16 = mybir.dt.bfloat16
F32 = mybir.dt.float32
F32R = mybir.dt.float32r
NBF = ml_dtypes.bfloat16

S = 2048
D = 2048
H = 32
G = 8
HD = 64
NCORE = 8
NHL = H // NCORE          # 4 q heads per core
EL = NHL * HD             # 256 local q (and gate, and ctx) features
QK = EL + HD              # 320: q + k features
QKV = QK + HD             # 384: q + k + v
P = 128
NS = S // P               # 16 s-tiles
ND = D // P               # 16 d-tiles
SQ = 512
NSQ = S // SQ             # 4 sq slices
NB = QK // HD             # 5 (hd,) blocks in the q|k strip
SCALE = HD ** -0.5
EPS = 1e-6

WAVES = [list(range(0, 6)), list(range(6, 12)), list(range(12, 16))]


def _v(ap, dims, extra_offset=0):
    """Reshape the free dims of a 2D AP into `dims` ([step, count] pairs),
    keeping the partition dim."""
    return bass.AP(
        tensor=ap.tensor,
        offset=ap.offset + extra_offset,
        ap=[list(ap.ap[0])] + [list(d) for d in dims],
    )


def build_nc():
    nc = bacc.Bacc("TRN2", target_bir_lowering=False, debug=False,
                   num_devices=NCORE)

    xt = nc.dram_tensor("xt", [D, S], BF16, kind="ExternalInput").ap()
    wqkv = nc.dram_tensor("wqkv", [D, QKV], BF16, kind="ExternalInput").ap()
    wg = nc.dram_tensor("wg", [D, EL], BF16, kind="ExternalInput").ap()
    wo = nc.dram_tensor("wo", [EL, D], BF16, kind="ExternalInput").ap()
    cos5 = nc.dram_tensor("cos5", [S, QK], BF16, kind="ExternalInput").ap()
    sin5 = nc.dram_tensor("sin5", [S, QK], BF16, kind="ExternalInput").ap()
    tri = nc.dram_tensor("tri", [P, P], BF16, kind="ExternalInput").ap()
    indad = nc.dram_tensor("inda", [1, P], F32R, kind="ExternalInput").ap()
    indbd = nc.dram_tensor("indb", [1, P], F32R, kind="ExternalInput").ap()
    out = nc.dram_tensor("out", [S, D], BF16, kind="ExternalOutput").ap()

    AF = mybir.ActivationFunctionType
    ALU = mybir.AluOpType

    with tile.TileContext(nc) as tc:
        with (
            tc.tile_pool(name="persist", bufs=1) as pp,
            tc.tile_pool(name="work", bufs=2) as wp,
            tc.tile_pool(name="stats", bufs=2) as sp,
            tc.tile_pool(name="probs", bufs=3) as prp,
            tc.tile_pool(name="rbp", bufs=2) as rp,
            tc.tile_pool(name="outc", bufs=4) as ocp,
        ):
            # ---- DMA loads (emission order = arrival order) ----
            wgs = []
            for i in range(ND):
                t = pp.tile([P, EL], BF16, tag=f"wg{i}", name=f"wg{i}")
                nc.sync.dma_start(out=t, in_=wg[i * P:(i + 1) * P, :])
                wgs.append(t)
            xts, wqkvs = [], []
            for i in range(ND):
                t = pp.tile([P, S], BF16, tag=f"xt{i}", name=f"xt{i}")
                nc.sync.dma_start(out=t, in_=xt[i * P:(i + 1) * P, :])
                xts.append(t)
                t2 = pp.tile([P, QKV], BF16, tag=f"wqkv{i}", name=f"wqkv{i}")
                nc.sync.dma_start(out=t2, in_=wqkv[i * P:(i + 1) * P, :])
                wqkvs.append(t2)
            coss, sins = [], []
            for j in range(NS):
                tc_ = pp.tile([P, QK], BF16, tag=f"cos{j}", name=f"cos{j}")
                nc.sync.dma_start(out=tc_, in_=cos5[j * P:(j + 1) * P, :])
                coss.append(tc_)
                ts_ = pp.tile([P, QK], BF16, tag=f"sin{j}", name=f"sin{j}")
                nc.sync.dma_start(out=ts_, in_=sin5[j * P:(j + 1) * P, :])
                sins.append(ts_)
            tri_sb = pp.tile([P, P], BF16, tag="tri", name="tri")
            nc.sync.dma_start(out=tri_sb, in_=tri)
            wos = []
            for e in range(2):
                t = pp.tile([P, D], BF16, tag=f"wo{e}", name=f"wo{e}")
                nc.sync.dma_start(out=t, in_=wo[e * P:(e + 1) * P, :])
                wos.append(t)

            ident = pp.tile([P, P], BF16, tag="ident", name="ident")
            make_identity(nc, ident)
            # den-broadcast weights (rb = 1/(2*den) = 0.5/den):
            # indA = 2.0 on cols 0:64, indB = 2.0 on cols 64:128
            indA = pp.tile([1, P], F32R, tag="indA", name="indA")
            nc.sync.dma_start(out=indA, in_=indad)
            indB = pp.tile([1, P], F32R, tag="indB", name="indB")
            nc.sync.dma_start(out=indB, in_=indbd)

            # persistent intermediates
            q2 = [[pp.tile([P, SQ], BF16, tag=f"q2_{p}_{q}", name=f"q2_{p}_{q}")
                   for q in range(NSQ)] for p in range(2)]
            kts2 = [pp.tile([P, SQ], BF16, tag=f"kt{q}", name=f"kt{q}")
                    for q in range(NSQ)]
            vs = [pp.tile([P, HD + 1], BF16, tag=f"v{j}", name=f"v{j}")
                  for j in range(NS)]
            gus = [[pp.tile([P, SQ], BF16, tag=f"gu{p}_{q}", name=f"gu{p}_{q}")
                    for q in range(NSQ)] for p in range(2)]
            ctxg2 = [[pp.tile([P, SQ], BF16, tag=f"cg{p}_{q}", name=f"cg{p}_{q}")
                      for q in range(NSQ)] for p in range(2)]
            for j in range(NS):
                nc.vector.memset(vs[j][:, HD:HD + 1], 1.0)

            # ---- phase G: gate projection, i-outer over 8 banks ----
            with tc.tile_pool(name="pg", bufs=1, space="PSUM") as pg:
                psg = [pg.tile([P, SQ], F32, tag="g", bufs=8, name=f"psg{u}")
                       for u in range(8)]
                for i in range(ND):
                    for u in range(8):
                        p_, q_ = u // NSQ, u % NSQ
                        nc.tensor.matmul(
                            psg[u], wgs[i][:, p_ * P:(p_ + 1) * P],
                            xts[i][:, q_ * SQ:(q_ + 1) * SQ],
                            start=(i == 0), stop=(i == ND - 1))
                for u in range(8):
                    p_, q_ = u // NSQ, u % NSQ
                    # u = 1 + tanh(g/2) = 2*sigmoid(g)
                    nc.scalar.activation(gus[p_][q_], psg[u], AF.Tanh,
                                         scale=0.5)
                    nc.vector.tensor_scalar_add(gus[p_][q_], gus[p_][q_], 1.0)

            # ---- phase Q: qkv projection in i-outer waves + norm/rope ----
            qkr_tiles = [None] * NS

            def emit_transposes(wave):
                for j in wave:
                    jq, jc = j // 4, (j % 4) * P
                    tp = pq.tile([P, 3 * P], BF16, tag="tp", bufs=2, name="tp")
                    nc.tensor.transpose(tp[:, 0:P],
                                        qkr_tiles[j][:, 0:P], ident)
                    nc.tensor.transpose(tp[:, P:2 * P],
                                        qkr_tiles[j][:, P:2 * P], ident)
                    nc.tensor.transpose(tp[:HD, 2 * P:3 * P],
                                        qkr_tiles[j][:, 2 * P:2 * P + HD],
                                        ident)
                    nc.vector.tensor_copy(out=q2[0][jq][:, jc:jc + P],
                                          in_=tp[:, 0:P])
                    nc.vector.tensor_copy(out=q2[1][jq][:, jc:jc + P],
                                          in_=tp[:, P:2 * P])
                    nc.vector.tensor_copy(out=kts2[jq][0:HD, jc:jc + P],
                                          in_=tp[:HD, 2 * P:3 * P])
                    nc.vector.tensor_copy(out=kts2[jq][HD:2 * HD, jc:jc + P],
                                          in_=tp[:HD, 2 * P:3 * P])

            with tc.tile_pool(name="pq", bufs=1, space="PSUM") as pq:
                for w, wave in enumerate(WAVES):
                    W = len(wave)
                    ps_w = [pq.tile([P, QKV], F32, tag="qkv", bufs=6,
                                    name=f"psq{j}") for j in wave]
                    for i in range(ND):
                        for jj, j in enumerate(wave):
                            nc.tensor.matmul(
                                ps_w[jj], xts[i][:, j * P:(j + 1) * P],
                                wqkvs[i],
                                start=(i == 0), stop=(i == ND - 1))
                    # evacuate psums + per-j sum of squares
                    ssw = sp.tile([P, 5 * W], F32, tag="ssw", name="ssw")
                    qks = []
                    for jj, j in enumerate(wave):
                        sqr = wp.tile([P, QK], F32, tag="sqr", bufs=3,
                                      name="sqr")
                        nc.scalar.activation(sqr, ps_w[jj][:, :QK], AF.Square)
                        qk_sb = wp.tile([P, QK], F32, tag="qk", bufs=6,
                                        name="qk")
                        nc.vector.tensor_copy(out=qk_sb, in_=ps_w[jj][:, :QK])
                        nc.vector.tensor_copy(out=vs[j][:, :HD],
                                              in_=ps_w[jj][:, QK:QKV])
                        nc.vector.tensor_reduce(
                            ssw[:, 5 * jj:5 * jj + 5],
                            _v(sqr, [[HD, NB], [1, HD]]),
                            axis=mybir.AxisListType.X, op=ALU.add)
                        qks.append(qk_sb)
                    # batched rsqrt: quadratic minimax seed + 3 Newton steps
                    nw = 5 * W
                    m = sp.tile([P, nw], F32, tag="m", name="m")
                    nc.vector.tensor_scalar(m, ssw, 1.0 / HD, EPS,
                                            ALU.mult, ALU.add)
                    mc = sp.tile([P, nw], F32, tag="mc", name="mc")
                    nc.vector.tensor_scalar(mc, m, 5.5, 0.45,
                                            ALU.min, ALU.max)
                    m2 = sp.tile([P, nw], F32, tag="m2", name="m2")
                    nc.vector.tensor_mul(m2, mc, mc)
                    lin = sp.tile([P, nw], F32, tag="lin", name="lin")
                    nc.vector.tensor_scalar(lin, mc, -0.48330447, 1.51774376,
                                            ALU.mult, ALU.add)
                    y = sp.tile([P, nw], F32, tag="y", name="y")
                    nc.vector.scalar_tensor_tensor(y, m2, 0.0534932, lin,
                                                   ALU.mult, ALU.add)
                    yt = sp.tile([P, nw], F32, tag="yt", name="yt")
                    for _ in range(3):
                        nc.vector.tensor_mul(yt, y, y)
                        nc.vector.tensor_mul(yt, yt, m)
                        nc.vector.tensor_scalar(yt, yt, -0.5, 1.5,
                                                ALU.mult, ALU.add)
                        nc.vector.tensor_mul(y, y, yt)
                    # normalize + rope per j
                    for jj, j in enumerate(wave):
                        qkn = wp.tile([P, QK], F32, tag="qkn", name="qkn")
                        nc.vector.tensor_mul(
                            _v(qkn, [[HD, NB], [1, HD]]),
                            _v(qks[jj], [[HD, NB], [1, HD]]),
                            _v(y[:, 5 * jj:5 * jj + 5], [[1, NB], [0, HD]]))
                        t1 = wp.tile([P, QK], F32, tag="t1", name="t1")
                        nc.vector.tensor_mul(t1, qkn, coss[j])
                        t2 = wp.tile([P, QK], F32, tag="t2", name="t2")
                        nc.vector.tensor_mul(
                            _v(t2, [[HD, NB], [32, 2], [1, 32]]),
                            _v(qkn, [[HD, NB], [-32, 2], [1, 32]],
                               extra_offset=32),
                            _v(sins[j][:, :], [[HD, NB], [32, 2], [1, 32]]))
                        qkr = wp.tile([P, QK], BF16, tag="qkr", bufs=8,
                                      name="qkr")
                        nc.vector.tensor_add(qkr, t1, t2)
                        qkr_tiles[j] = qkr
                    if w > 0:
                        emit_transposes(WAVES[w - 1])
                emit_transposes(WAVES[-1])

            # ---- phase A: attention + output projection ----
            with tc.tile_pool(name="pa", bufs=1, space="PSUM") as pa:
                for q in range(NSQ):
                    for p in range(2):
                        nks = 4 * q + 4
                        ctxA = pa.tile([HD + 1, SQ], F32, tag="ctx", bufs=2,
                                       name="ctxA")
                        ctxB = pa.tile([HD + 1, SQ], F32, tag="ctx", bufs=2,
                                       name="ctxB")
                        prev = None
                        for jk in range(nks):
                            dlt = jk - 4 * q
                            c0 = max(dlt, 0) * P
                            jqk, kc = jk // 4, (jk % 4) * P
                            psS = pa.tile([P, 2 * SQ], F32, tag="s", bufs=2,
                                          name="psS")
                            nc.tensor.matmul(
                                psS[:, c0:SQ],
                                kts2[jqk][0:HD, kc:kc + P],
                                q2[p][q][0:HD, c0:SQ],
                                start=True, stop=True)
                            nc.tensor.matmul(
                                psS[:, SQ + c0:2 * SQ],
                                kts2[jqk][HD:2 * HD, kc:kc + P],
                                q2[p][q][HD:P, c0:SQ],
                                start=True, stop=True)
                            pr = prp.tile([P, 2 * SQ], BF16, tag="pr",
                                          name="pr")
                            if c0:
                                nc.scalar.activation(
                                    _v(pr, [[SQ, 2], [1, SQ - c0]],
                                       extra_offset=c0),
                                    _v(psS, [[SQ, 2], [1, SQ - c0]],
                                       extra_offset=c0),
                                    AF.Exp, scale=SCALE)
                            else:
                                nc.scalar.activation(pr, psS, AF.Exp,
                                                     scale=SCALE)
                            if dlt >= 0:
                                nc.vector.tensor_mul(
                                    pr[:, c0:c0 + P], pr[:, c0:c0 + P],
                                    tri_sb)
                                nc.vector.tensor_mul(
                                    pr[:, SQ + c0:SQ + c0 + P],
                                    pr[:, SQ + c0:SQ + c0 + P], tri_sb)
                            if prev is not None:
                                pjk, ppr, pc0 = prev
                                nc.tensor.matmul(
                                    ctxA[:, pc0:], vs[pjk], ppr[:, pc0:SQ],
                                    start=(pjk == 0), stop=False)
                                nc.tensor.matmul(
                                    ctxB[:, pc0:], vs[pjk],
                                    ppr[:, SQ + pc0:2 * SQ],
                                    start=(pjk == 0), stop=False)
                            prev = (jk, pr, c0)
                        pjk, ppr, pc0 = prev
                        nc.tensor.matmul(
                            ctxA[:, pc0:], vs[pjk], ppr[:, pc0:SQ],
                            start=(pjk == 0), stop=True)
                        nc.tensor.matmul(
                            ctxB[:, pc0:], vs[pjk], ppr[:, SQ + pc0:2 * SQ],
                            start=(pjk == 0), stop=True)
                        # rb = 0.5/den via PE broadcast + 128-lane reciprocal
                        denAB = rp.tile([1, 2 * SQ], F32R, tag="den",
                                        name="denAB")
                        nc.vector.tensor_copy(out=denAB[:, 0:SQ],
                                              in_=ctxA[HD:HD + 1, :])
                        nc.vector.tensor_copy(out=denAB[:, SQ:2 * SQ],
                                              in_=ctxB[HD:HD + 1, :])
                        ps_rb = pa.tile([P, SQ], F32, tag="mm", bufs=2,
                                        name="ps_rb")
                        nc.tensor.matmul(ps_rb, indA, denAB[:, 0:SQ],
                                         start=True, stop=False)
                        nc.tensor.matmul(ps_rb, indB, denAB[:, SQ:2 * SQ],
                                         start=False, stop=True)
                        rb = rp.tile([P, SQ], F32, tag="rb", name="rb")
                        nc.vector.reciprocal_approx_fast(out=rb, in_=ps_rb)
                        # ctxg = ctx * u * (0.5/den)
                        tmpc = rp.tile([P, SQ], F32, tag="tmpc", name="tmpc")
                        nc.vector.tensor_mul(tmpc[0:HD, :], ctxA[0:HD, :],
                                             gus[p][q][0:HD, :])
                        nc.vector.tensor_mul(tmpc[HD:P, :], ctxB[0:HD, :],
                                             gus[p][q][HD:P, :])
                        nc.vector.tensor_mul(ctxg2[p][q], tmpc, rb)
                    # output projection for this q-slice
                    for j in range(4 * q, 4 * q + 4):
                        jc = (j % 4) * P
                        for n in range(NSQ):
                            pso = pa.tile([P, SQ], F32, tag="mm", bufs=2,
                                          name="pso")
                            nc.tensor.matmul(
                                pso, ctxg2[0][q][:, jc:jc + P],
                                wos[0][:, n * SQ:(n + 1) * SQ],
                                start=True, stop=False)
                            nc.tensor.matmul(
                                pso, ctxg2[1][q][:, jc:jc + P],
                                wos[1][:, n * SQ:(n + 1) * SQ],
                                start=False, stop=True)
                            oc = ocp.tile([P, SQ], BF16, tag="oc", name="oc")
                            nc.vector.tensor_copy(out=oc, in_=pso)
                            nc.sync.dma_start(
                                out=out[j * P:(j + 1) * P,
                                        n * SQ:(n + 1) * SQ],
                                in_=oc)

    nc.compile()
    return nc


def prep_inputs(x, cos, sin, Wq, Wk, Wv, Wo, q_norm_w, k_norm_w):
    """Host-side shard + layout prep. Returns per-core input maps."""
    xtn = np.ascontiguousarray(x.reshape(S, D).T).astype(NBF)

    # rope tables with (1 + norm_w) folded in, k-block appended, and the
    # sin first-half pre-negated (so rope is out = q*cos5 + rot(q)*sin5
    # with rot(q) = [q2, q1])
    half = HD // 2
    wq1 = (1.0 + q_norm_w).astype(np.float32)
    wk1 = (1.0 + k_norm_w).astype(np.float32)

    def rotw(w):
        return np.concatenate([w[half:], w[:half]])

    sin_m = sin.copy()
    sin_m[:, :half] = -sin_m[:, :half]
    cos_q = cos * wq1
    cos_k = cos * wk1
    sin_q = sin_m * rotw(wq1)
    sin_k = sin_m * rotw(wk1)
    cos5 = np.concatenate([np.tile(cos_q, (1, NHL)), cos_k], axis=1)
    sin5 = np.concatenate([np.tile(sin_q, (1, NHL)), sin_k], axis=1)
    cos5 = np.ascontiguousarray(cos5).astype(NBF)
    sin5 = np.ascontiguousarray(sin5).astype(NBF)

    tri = np.triu(np.ones((P, P), dtype=np.float32)).astype(NBF)  # p<=f
    inda = np.zeros((1, P), dtype=np.float32)
    inda[0, :HD] = 2.0
    indb = np.zeros((1, P), dtype=np.float32)
    indb[0, HD:] = 2.0

    Wqh = Wq.reshape(H, 2 * HD, D)
    in_maps = []
    for c in range(NCORE):
        hs = slice(NHL * c, NHL * (c + 1))
        wq_c = Wqh[hs, :HD, :].reshape(EL, D)       # q rows, 4 heads
        wgt_c = Wqh[hs, HD:, :].reshape(EL, D)      # gate rows
        wk_c = Wk[HD * c:HD * (c + 1), :]
        wv_c = Wv[HD * c:HD * (c + 1), :]
        wqkv_c = np.concatenate([wq_c, wk_c, wv_c], axis=0)  # [384, D]
        in_maps.append({
            "xt": xtn,
            "wqkv": np.ascontiguousarray(wqkv_c.T).astype(NBF),
            "wg": np.ascontiguousarray(wgt_c.T).astype(NBF),
            "wo": np.ascontiguousarray(
                Wo[:, EL * c:EL * (c + 1)].T).astype(NBF),
            "cos5": cos5,
            "sin5": sin5,
            "tri": tri,
            "inda": inda,
            "indb": indb,
        })
    return in_maps


_NC_CACHE = {}


def get_nc():
    if "nc" not in _NC_CACHE:
        _NC_CACHE["nc"] = build_nc()
    return _NC_CACHE["nc"]


def run(in_maps, trace=False, **kw):
    nc = get_nc()
    return run_bass_kernel_spmd(nc, in_maps, list(range(NCORE)),
                                trace=trace, **kw)


def kernel(x, mask, cos, sin, Wq, Wk, Wv, Wo, q_norm_w, k_norm_w):
    in_maps = prep_inputs(np.asarray(x, dtype=np.float32), np.asarray(cos),
                          np.asarray(sin), np.asarray(Wq), np.asarray(Wk),
                          np.asarray(Wv), np.asarray(Wo),
                          np.asarray(q_norm_w), np.asarray(k_norm_w))
    res = run(in_maps)
    acc = np.zeros((S, D), dtype=np.float32)
    for r in res.results:
        acc += np.asarray(r["out"], dtype=np.float32)
    return acc.reshape(1, S, D)


# revision 20
# speedup vs baseline: 2.1421x; 1.0552x over previous
"""GQA attention block (B=1, S=2048, D=2048, H=32, G=8, HD=64) on 8 trn2 cores.

Sharding: tensor-parallel over heads/KV-groups. Core c owns q-heads
4c..4c+3 and KV group c. Wq/Wk/Wv column-parallel, Wo row-parallel;
each core computes a partial [S, D] output, host sums the 8 partials.

v3 schedule: qkv projection in 3 i-outer waves of 6 psum banks (wave 0
paced by the x DMA stream), RMSNorm Newton chain batched per wave, PE
transposes of wave w behind wave w+1's matmuls. Attention packs the two
heads of a pair via row tiling (K=64 at base partitions 0/64); the
causal mask is applied with an accumulating -400 mask matmul on the
diagonal blocks (exp then underflows to 0) so the ACT->PE chain never
routes through DVE; ctx matmuls trail scores by 2 steps. The gate
projection is interleaved per-unit inside the attention phase to keep
the PE stream dense (HAM warm) while exp paces. Softmax denominator is
PE-broadcast (f32r) then reciprocal'd on 128 lanes. Loads are split
across the sync (xt/wqkv) and gpsimd (tables/weights) DMA queues; psum
evacuations of the qkv phase run on the scalar engine.
"""

import numpy as np
import ml_dtypes

import concourse.bass as bass
import concourse.tile as tile
from concourse import bacc, mybir
from concourse.bass_utils import run_bass_kernel_spmd
from concourse.masks import make_identity

BF16 = mybir.dt.bfloat16
F32 = mybir.dt.float32
F32R = mybir.dt.float32r
NBF = ml_dtypes.bfloat16

S = 2048
D = 2048
H = 32
G = 8
HD = 64
NCORE = 8
NHL = H // NCORE          # 4 q heads per core
EL = NHL * HD             # 256 local q (and gate, and ctx) features
QK = EL + HD              # 320: q + k features
QKV = QK + HD             # 384: q + k + v
P = 128
NS = S // P               # 16 s-tiles
ND = D // P               # 16 d-tiles
SQ = 512
NSQ = S // SQ             # 4 sq slices
NB = QK // HD             # 5 (hd,) blocks in the q|k strip
SCALE = HD ** -0.5
EPS = 1e-6
MBIG = 400.0              # causal mask bias: exp(scale*(s-400)) ~ 0

WAVES = [list(range(0, 6)), list(range(6, 12)), list(range(12, 16))]


def _v(ap, dims, extra_offset=0):
    """Reshape the free dims of a 2D AP into `dims` ([step, count] pairs),
    keeping the partition dim."""
    return bass.AP(
        tensor=ap.tensor,
        offset=ap.offset + extra_offset,
        ap=[list(ap.ap[0])] + [list(d) for d in dims],
    )


def build_nc():
    nc = bacc.Bacc("TRN2", target_bir_lowering=False, debug=False,
                   num_devices=NCORE)

    xt = nc.dram_tensor("xt", [D, S], BF16, kind="ExternalInput").ap()
    wqkv = nc.dram_tensor("wqkv", [D, QKV], BF16, kind="ExternalInput").ap()
    wg = nc.dram_tensor("wg", [D, EL], BF16, kind="ExternalInput").ap()
    wo = nc.dram_tensor("wo", [EL, D], BF16, kind="ExternalInput").ap()
    cos5 = nc.dram_tensor("cos5", [S, QK], BF16, kind="ExternalInput").ap()
    sin5 = nc.dram_tensor("sin5", [S, QK], BF16, kind="ExternalInput").ap()
    negidd = nc.dram_tensor("negid", [P, P], BF16, kind="ExternalInput").ap()
    stepmd = nc.dram_tensor("stepm", [P, P], BF16, kind="ExternalInput").ap()
    indad = nc.dram_tensor("inda", [1, P], F32R, kind="ExternalInput").ap()
    indbd = nc.dram_tensor("indb", [1, P], F32R, kind="ExternalInput").ap()
    out = nc.dram_tensor("out", [S, D], BF16, kind="ExternalOutput").ap()

    AF = mybir.ActivationFunctionType
    ALU = mybir.AluOpType

    with tile.TileContext(nc) as tc:
        with (
            tc.tile_pool(name="persist", bufs=1) as pp,
            tc.tile_pool(name="work", bufs=2) as wp,
            tc.tile_pool(name="stats", bufs=2) as sp,
            tc.tile_pool(name="probs", bufs=4) as prp,
            tc.tile_pool(name="rbp", bufs=2) as rp,
            tc.tile_pool(name="outc", bufs=4) as ocp,
        ):
            # ---- identity first (gpsimd compute, before its DMA backlog) ----
            ident = pp.tile([P, P], BF16, tag="ident", name="ident")
            make_identity(nc, ident)

            # ---- DMA loads: xt+wqkv on the sync queue (paces wave 0),
            # rope tables + consts on the scalar queue, wg/wo on gpsimd ----
            xts, wqkvs = [], []
            for i in range(ND):
                t = pp.tile([P, S], BF16, tag=f"xt{i}", name=f"xt{i}")
                nc.sync.dma_start(out=t, in_=xt[i * P:(i + 1) * P, :])
                xts.append(t)
                t2 = pp.tile([P, QKV], BF16, tag=f"wqkv{i}", name=f"wqkv{i}")
                nc.sync.dma_start(out=t2, in_=wqkv[i * P:(i + 1) * P, :])
                wqkvs.append(t2)
            negid = pp.tile([P, P], BF16, tag="negid", name="negid")
            nc.scalar.dma_start(out=negid, in_=negidd)
            stepm = pp.tile([P, P], BF16, tag="stepm", name="stepm")
            nc.scalar.dma_start(out=stepm, in_=stepmd)
            indA = pp.tile([1, P], F32R, tag="indA", name="indA")
            nc.scalar.dma_start(out=indA, in_=indad)
            indB = pp.tile([1, P], F32R, tag="indB", name="indB")
            nc.scalar.dma_start(out=indB, in_=indbd)
            coss, sins = [], []
            for j in range(NS):
                tc_ = pp.tile([P, QK], BF16, tag=f"cos{j}", name=f"cos{j}")
                nc.scalar.dma_start(out=tc_, in_=cos5[j * P:(j + 1) * P, :])
                coss.append(tc_)
                ts_ = pp.tile([P, QK], BF16, tag=f"sin{j}", name=f"sin{j}")
                nc.scalar.dma_start(out=ts_, in_=sin5[j * P:(j + 1) * P, :])
                sins.append(ts_)
            wgs = []
            for i in range(ND):
                t = pp.tile([P, EL], BF16, tag=f"wg{i}", name=f"wg{i}")
                nc.gpsimd.dma_start(out=t, in_=wg[i * P:(i + 1) * P, :])
                wgs.append(t)
            wos = []
            for e in range(2):
                t = pp.tile([P, D], BF16, tag=f"wo{e}", name=f"wo{e}")
                nc.gpsimd.dma_start(out=t, in_=wo[e * P:(e + 1) * P, :])
                wos.append(t)

            # persistent intermediates
            # q2[q]: cols [0:SQ] = pair0 (rows h0|h1 qT), [SQ:2SQ] = pair1
            q2 = [pp.tile([P, 2 * SQ], BF16, tag=f"q2_{q}", name=f"q2_{q}")
                  for q in range(NSQ)]
            kts2 = [pp.tile([P, SQ], BF16, tag=f"kt{q}", name=f"kt{q}")
                    for q in range(NSQ)]
            vs = [pp.tile([P, HD + 1], BF16, tag=f"v{j}", name=f"v{j}")
                  for j in range(NS)]
            gus = [[pp.tile([P, SQ], BF16, tag=f"gu{p}_{q}", name=f"gu{p}_{q}")
                    for q in range(NSQ)] for p in range(2)]
            ctxg2 = [[pp.tile([P, SQ], BF16, tag=f"cg{p}_{q}", name=f"cg{p}_{q}")
                      for q in range(NSQ)] for p in range(2)]
            for j in range(NS):
                nc.vector.memset(vs[j][:, HD:HD + 1], 1.0)

            # ---- phase Q: qkv projection in i-outer waves + norm/rope ----
            qkr_tiles = [None] * NS

            def emit_transposes(wave):
                for j in wave:
                    jq, jc = j // 4, (j % 4) * P
                    tp = pq.tile([P, 3 * P], BF16, tag="tp", bufs=2, name="tp")
                    nc.tensor.transpose(tp[:, 0:P],
                                        qkr_tiles[j][:, 0:P], ident)
                    nc.tensor.transpose(tp[:, P:2 * P],
                                        qkr_tiles[j][:, P:2 * P], ident)
                    nc.tensor.transpose(tp[:HD, 2 * P:3 * P],
                                        qkr_tiles[j][:, 2 * P:2 * P + HD],
                                        ident)
                    # both head-pairs in one copy: dst cols {jc, SQ+jc}
                    nc.vector.tensor_copy(
                        out=_v(q2[jq], [[SQ, 2], [1, P]], extra_offset=jc),
                        in_=_v(tp, [[P, 2], [1, P]]))
                    nc.vector.tensor_copy(out=kts2[jq][0:HD, jc:jc + P],
                                          in_=tp[:HD, 2 * P:3 * P])
                    nc.vector.tensor_copy(out=kts2[jq][HD:2 * HD, jc:jc + P],
                                          in_=tp[:HD, 2 * P:3 * P])

            with tc.tile_pool(name="pq", bufs=1, space="PSUM") as pq:
                for w, wave in enumerate(WAVES):
                    W = len(wave)
                    ps_w = [pq.tile([P, QKV], F32, tag="qkv", bufs=6,
                                    name=f"psq{j}") for j in wave]
                    for i in range(ND):
                        for jj, j in enumerate(wave):
                            nc.tensor.matmul(
                                ps_w[jj], xts[i][:, j * P:(j + 1) * P],
                                wqkvs[i],
                                start=(i == 0), stop=(i == ND - 1))
                    # evacuate psums (scalar engine) + per-j sum of squares
                    ssw = sp.tile([P, 5 * W], F32, tag="ssw", name="ssw")
                    qks = []
                    for jj, j in enumerate(wave):
                        sqr = wp.tile([P, QK], F32, tag="sqr", bufs=3,
                                      name="sqr")
                        nc.scalar.activation(sqr, ps_w[jj][:, :QK], AF.Square)
                        qk_sb = wp.tile([P, QK], F32, tag="qk", bufs=6,
                                        name="qk")
                        nc.scalar.copy(qk_sb, ps_w[jj][:, :QK])
                        nc.scalar.copy(vs[j][:, :HD], ps_w[jj][:, QK:QKV])
                        nc.vector.tensor_reduce(
                            ssw[:, 5 * jj:5 * jj + 5],
                            _v(sqr, [[HD, NB], [1, HD]]),
                            axis=mybir.AxisListType.X, op=ALU.add)
                        qks.append(qk_sb)
                    # batched rsqrt: quadratic minimax seed + 3 Newton steps
                    nw = 5 * W
                    m = sp.tile([P, nw], F32, tag="m", name="m")
                    nc.vector.tensor_scalar(m, ssw, 1.0 / HD, EPS,
                                            ALU.mult, ALU.add)
                    mc = sp.tile([P, nw], F32, tag="mc", name="mc")
                    nc.vector.tensor_scalar(mc, m, 5.5, 0.45,
                                            ALU.min, ALU.max)
                    m2 = sp.tile([P, nw], F32, tag="m2", name="m2")
                    nc.vector.tensor_mul(m2, mc, mc)
                    lin = sp.tile([P, nw], F32, tag="lin", name="lin")
                    nc.vector.tensor_scalar(lin, mc, -0.48330447, 1.51774376,
                                            ALU.mult, ALU.add)
                    y = sp.tile([P, nw], F32, tag="y", name="y")
                    nc.vector.scalar_tensor_tensor(y, m2, 0.0534932, lin,
                                                   ALU.mult, ALU.add)
                    yt = sp.tile([P, nw], F32, tag="yt", name="yt")
                    for _ in range(3):
                        nc.vector.tensor_mul(yt, y, y)
                        nc.vector.tensor_mul(yt, yt, m)
                        nc.vector.tensor_scalar(yt, yt, -0.5, 1.5,
                                                ALU.mult, ALU.add)
                        nc.vector.tensor_mul(y, y, yt)
                    # normalize + rope per j (bf16 rope muls, 2x DVE rate)
                    for jj, j in enumerate(wave):
                        qkn = wp.tile([P, QK], BF16, tag="qkn", name="qkn")
                        nc.vector.tensor_mul(
                            _v(qkn, [[HD, NB], [1, HD]]),
                            _v(qks[jj], [[HD, NB], [1, HD]]),
                            _v(y[:, 5 * jj:5 * jj + 5], [[1, NB], [0, HD]]))
                        t1 = wp.tile([P, QK], BF16, tag="t1", name="t1")
                        nc.vector.tensor_mul(t1, qkn, coss[j])
                        t2 = wp.tile([P, QK], BF16, tag="t2", name="t2")
                        nc.vector.tensor_mul(
                            _v(t2, [[HD, NB], [32, 2], [1, 32]]),
                            _v(qkn, [[HD, NB], [-32, 2], [1, 32]],
                               extra_offset=32),
                            _v(sins[j][:, :], [[HD, NB], [32, 2], [1, 32]]))
                        qkr = wp.tile([P, QK], BF16, tag="qkr", bufs=8,
                                      name="qkr")
                        nc.vector.tensor_add(qkr, t1, t2)
                        qkr_tiles[j] = qkr
                    if w > 0:
                        emit_transposes(WAVES[w - 1])
                emit_transposes(WAVES[-1])

            # ---- phase A: gate + attention + output projection ----
            with tc.tile_pool(name="pa", bufs=1, space="PSUM") as pa:
                oidx = 0
                for q in range(NSQ):
                    for p in range(2):
                        # gate projection for this unit (dense PE filler)
                        psg = pa.tile([P, SQ], F32, tag="mm", bufs=2,
                                      name="psg")
                        for i in range(ND):
                            nc.tensor.matmul(
                                psg, wgs[i][:, p * P:(p + 1) * P],
                                xts[i][:, q * SQ:(q + 1) * SQ],
                                start=(i == 0), stop=(i == ND - 1))
                        nc.scalar.activation(gus[p][q], psg, AF.Tanh,
                                             scale=0.5)
                        nc.vector.tensor_scalar_add(gus[p][q], gus[p][q], 1.0)

                        # attention unit: heads (2p, 2p+1)
                        nks = 4 * q + 4
                        ctxA = pa.tile([HD + 1, SQ], F32, tag="ctx", bufs=2,
                                       name="ctxA")
                        ctxB = pa.tile([HD + 1, SQ], F32, tag="ctx", bufs=2,
                                       name="ctxB")

                        def emit_ctx(ent):
                            pjk, ppr, pc0 = ent
                            nc.tensor.matmul(
                                ctxA[:, pc0:], vs[pjk], ppr[:, pc0:SQ],
                                start=(pjk == 0), stop=(pjk == nks - 1))
                            nc.tensor.matmul(
                                ctxB[:, pc0:], vs[pjk],
                                ppr[:, SQ + pc0:2 * SQ],
                                start=(pjk == 0), stop=(pjk == nks - 1))

                        pend = []
                        for jk in range(nks):
                            dlt = jk - 4 * q
                            c0 = max(dlt, 0) * P
                            jqk, kc = jk // 4, (jk % 4) * P
                            diag = dlt >= 0
                            psS = pa.tile([P, 2 * SQ], F32, tag="s", bufs=2,
                                          name="psS")
                            nc.tensor.matmul(
                                psS[:, c0:SQ],
                                kts2[jqk][0:HD, kc:kc + P],
                                q2[q][0:HD, p * SQ + c0:(p + 1) * SQ],
                                start=True, stop=not diag)
                            nc.tensor.matmul(
                                psS[:, SQ + c0:2 * SQ],
                                kts2[jqk][HD:2 * HD, kc:kc + P],
                                q2[q][HD:P, p * SQ + c0:(p + 1) * SQ],
                                start=True, stop=not diag)
                            if diag:
                                # causal mask: += -400 above the diagonal
                                nc.tensor.matmul(
                                    psS[:, c0:c0 + P], negid, stepm,
                                    start=False, stop=True)
                                nc.tensor.matmul(
                                    psS[:, SQ + c0:SQ + c0 + P], negid,
                                    stepm, start=False, stop=True)
                            pr = prp.tile([P, 2 * SQ], BF16, tag="pr",
                                          name="pr")
                            if c0:
                                nc.scalar.activation(
                                    _v(pr, [[SQ, 2], [1, SQ - c0]],
                                       extra_offset=c0),
                                    _v(psS, [[SQ, 2], [1, SQ - c0]],
                                       extra_offset=c0),
                                    AF.Exp, scale=SCALE)
                            else:
                                nc.scalar.activation(pr, psS, AF.Exp,
                                                     scale=SCALE)
                            pend.append((jk, pr, c0))
                            if len(pend) > 2:
                                emit_ctx(pend.pop(0))
                        for ent in pend:
                            emit_ctx(ent)
                        # rb = 0.5/den via PE broadcast + 128-lane reciprocal
                        denAB = rp.tile([1, 2 * SQ], F32R, tag="den",
                                        name="denAB")
                        nc.vector.tensor_copy(out=denAB[:, 0:SQ],
                                              in_=ctxA[HD:HD + 1, :])
                        nc.vector.tensor_copy(out=denAB[:, SQ:2 * SQ],
                                              in_=ctxB[HD:HD + 1, :])
                        ps_rb = pa.tile([P, SQ], F32, tag="mm", bufs=2,
                                        name="ps_rb")
                        nc.tensor.matmul(ps_rb, indA, denAB[:, 0:SQ],
                                         start=True, stop=False)
                        nc.tensor.matmul(ps_rb, indB, denAB[:, SQ:2 * SQ],
                                         start=False, stop=True)
                        rb = rp.tile([P, SQ], F32, tag="rb", name="rb")
                        nc.vector.reciprocal_approx_fast(out=rb, in_=ps_rb)
                        # ctxg = ctx * u * (0.5/den)
                        tmpc = rp.tile([P, SQ], F32, tag="tmpc", name="tmpc")
                        nc.vector.tensor_mul(tmpc[0:HD, :], ctxA[0:HD, :],
                                             gus[p][q][0:HD, :])
                        nc.vector.tensor_mul(tmpc[HD:P, :], ctxB[0:HD, :],
                                             gus[p][q][HD:P, :])
                        nc.vector.tensor_mul(ctxg2[p][q], tmpc, rb)
                    # output projection for this q-slice
                    for j in range(4 * q, 4 * q + 4):
                        jc = (j % 4) * P
                        for n in range(NSQ):
                            pso = pa.tile([P, SQ], F32, tag="mm", bufs=2,
                                          name="pso")
                            nc.tensor.matmul(
                                pso, ctxg2[0][q][:, jc:jc + P],
                                wos[0][:, n * SQ:(n + 1) * SQ],
                                start=True, stop=False)
                            nc.tensor.matmul(
                                pso, ctxg2[1][q][:, jc:jc + P],
                                wos[1][:, n * SQ:(n + 1) * SQ],
                                start=False, stop=True)
                            oc = ocp.tile([P, SQ], BF16, tag="oc", name="oc")
                            nc.vector.tensor_copy(out=oc, in_=pso)
                            eng = nc.sync if oidx % 2 == 0 else nc.gpsimd
                            eng.dma_start(
                                out=out[j * P:(j + 1) * P,
                                        n * SQ:(n + 1) * SQ],
                                in_=oc)
                            oidx += 1

    nc.compile()
    return nc


def prep_inputs(x, cos, sin, Wq, Wk, Wv, Wo, q_norm_w, k_norm_w):
    """Host-side shard + layout prep. Returns per-core input maps."""
    xtn = np.ascontiguousarray(x.reshape(S, D).T).astype(NBF)

    # rope tables with (1 + norm_w) folded in, k-block appended, and the
    # sin first-half pre-negated (so rope is out = q*cos5 + rot(q)*sin5
    # with rot(q) = [q2, q1])
    half = HD // 2
    wq1 = (1.0 + q_norm_w).astype(np.float32)
    wk1 = (1.0 + k_norm_w).astype(np.float32)

    def rotw(w):
        return np.concatenate([w[half:], w[:half]])

    sin_m = sin.copy()
    sin_m[:, :half] = -sin_m[:, :half]
    cos_q = cos * wq1
    cos_k = cos * wk1
    sin_q = sin_m * rotw(wq1)
    sin_k = sin_m * rotw(wk1)
    cos5 = np.concatenate([np.tile(cos_q, (1, NHL)), cos_k], axis=1)
    sin5 = np.concatenate([np.tile(sin_q, (1, NHL)), sin_k], axis=1)
    cos5 = np.ascontiguousarray(cos5).astype(NBF)
    sin5 = np.ascontiguousarray(sin5).astype(NBF)

    negid = (-MBIG * np.eye(P, dtype=np.float32)).astype(NBF)
    stepm = np.tril(np.ones((P, P), dtype=np.float32), -1).astype(NBF)
    inda = np.zeros((1, P), dtype=np.float32)
    inda[0, :HD] = 2.0
    indb = np.zeros((1, P), dtype=np.float32)
    indb[0, HD:] = 2.0

    Wqh = Wq.reshape(H, 2 * HD, D)
    in_maps = []
    for c in range(NCORE):
        hs = slice(NHL * c, NHL * (c + 1))
        wq_c = Wqh[hs, :HD, :].reshape(EL, D)       # q rows, 4 heads
        wgt_c = Wqh[hs, HD:, :].reshape(EL, D)      # gate rows
        wk_c = Wk[HD * c:HD * (c + 1), :]
        wv_c = Wv[HD * c:HD * (c + 1), :]
        wqkv_c = np.concatenate([wq_c, wk_c, wv_c], axis=0)  # [384, D]
        in_maps.append({
            "xt": xtn,
            "wqkv": np.ascontiguousarray(wqkv_c.T).astype(NBF),
            "wg": np.ascontiguousarray(wgt_c.T).astype(NBF),
            "wo": np.ascontiguousarray(
                Wo[:, EL * c:EL * (c + 1)].T).astype(NBF),
            "cos5": cos5,
            "sin5": sin5,
            "negid": negid,
            "stepm": stepm,
            "inda": inda,
            "indb": indb,
        })
    return in_maps


_NC_CACHE = {}


def get_nc():
    if "nc" not in _NC_CACHE:
        _NC_CACHE["nc"] = build_nc()
    return _NC_CACHE["nc"]


def run(in_maps, trace=False, **kw):
    nc = get_nc()
    return run_bass_kernel_spmd(nc, in_maps, list(range(NCORE)),
                                trace=trace, **kw)


def kernel(x, mask, cos, sin, Wq, Wk, Wv, Wo, q_norm_w, k_norm_w):
    in_maps = prep_inputs(np.asarray(x, dtype=np.float32), np.asarray(cos),
                          np.asarray(sin), np.asarray(Wq), np.asarray(Wk),
                          np.asarray(Wv), np.asarray(Wo),
                          np.asarray(q_norm_w), np.asarray(k_norm_w))
    res = run(in_maps)
    acc = np.zeros((S, D), dtype=np.float32)
    for r in res.results:
        acc += np.asarray(r["out"], dtype=np.float32)
    return acc.reshape(1, S, D)
